# revision 12
# baseline (speedup 1.0000x reference)
"""Bass/Trainium2 kernel for nn_NaryTreeLSTM (binary TreeLSTM over a complete
depth-16 tree, H=D=256, heap/level node order).

Sharding: data-parallel over 8 independent subtrees (core m owns the subtree
rooted at level-3 node m). The device computes levels 15..CUT per core in a
single F-layout (feature-on-partition, nodes-on-free) pipeline; the tiny top
of the tree (2^CUT-1 nodes) is finished on host during the gather/unshard
step (the cross-core combine has to leave the device at level 3 anyway, and
the top levels are latency-bound serial remnants on device).

v2 vs v1: all-bf16 matmul/elementwise datapath (error ~4e-3, well under the
2e-2 gate; halves HBM traffic and enables fast weight loads), Wf*x computed
once per node and shared between both forget gates via a fused
scalar_tensor_tensor on DVE, wide multi-bank ACTIVATEs (the scalar engine
has no instruction pipelining, so per-instruction fixed cost ~290ns
dominates), and both output-feature halves processed per chunk with single
2D-AP DVE ops in 2x bf16 mode. The N-layout tail + PE transposes of v1 are
gone entirely (CUT=11).

Node order per level is bit-reversed so the even/odd children of a
contiguous parent chunk are the first/second half of the child level.

Per node (children h_e,h_o / c_e,c_o; x = emb row):
  i = sig(Wi x + bi + Ui0 h_e + Ui1 h_o)      o, u analogous (u: tanh)
  f0 = sig(Wf x + bf + Uf0 h_e),  f1 = sig(Wf x + bf + Uf1 h_o)
  c = i*u + f0*c_e + f1*c_o ;  h = o * tanh(c)
"""

import os

import numpy as np

try:
    import concourse  # noqa: F401
except ImportError:  # pragma: no cover
    import sys

    sys.path.insert(0, "/opt/trn_rl_repo")

import ml_dtypes

import concourse.tile as tile
from concourse import bacc, mybir
from concourse.bass_utils import run_bass_kernel_spmd

F32 = mybir.dt.float32
BF16 = mybir.dt.bfloat16
AF = mybir.ActivationFunctionType
ALU = mybir.AluOpType
NPBF16 = ml_dtypes.bfloat16

DEPTH = 16
H = 256
P = 128
NCORES = 8
LTOP = DEPTH - 1
CUT = int(os.environ.get("TREELSTM_CUT", "11"))  # device: levels 15..CUT
CHUNK = 1024  # two PSUM banks per gate tile; one ACTIVATE per gate per chunk

N_L = {l: 1 << (l - 3) for l in range(CUT, LTOP + 1)}
NSLOT = sum(N_L.values())
OFF = {}
_o = 0
for _l in range(LTOP, CUT - 1, -1):
    OFF[_l] = _o
    _o += N_L[_l]
NOUT = N_L[CUT]
NBLK = (NSLOT + CHUNK - 1) // CHUNK

# weight table gate indices
G_I, G_O, G_U, G_F = 0, 1, 2, 3
U_I0, U_I1, U_O0, U_O1, U_U0, U_U1, U_F0, U_F1 = range(8)


def _bitrev(nbits):
    n = 1 << nbits
    r = np.zeros(n, dtype=np.int64)
    for j in range(n):
        v = 0
        for b in range(nbits):
            if j & (1 << b):
                v |= 1 << (nbits - 1 - b)
        r[j] = v
    return r


def _build_program():
    nc = bacc.Bacc("TRN2", target_bir_lowering=False, debug=False, num_devices=NCORES)
    xtb = nc.dram_tensor("xtb", [NBLK, P, 2, CHUNK], BF16, kind="ExternalInput").ap()
    wx = nc.dram_tensor("wx", [P, 4, 2, 2, P], BF16, kind="ExternalInput").ap()
    wu = nc.dram_tensor("wu", [P, 8, 2, 2, P], BF16, kind="ExternalInput").ap()
    bs = nc.dram_tensor("bs", [P, 4, 2], F32, kind="ExternalInput").ap()
    hc = nc.dram_tensor("hc", [P, 2, 2, NOUT], BF16, kind="ExternalOutput").ap()

    with tile.TileContext(nc) as tc:
        with (
            tc.tile_pool(name="const", bufs=1) as const,
            tc.tile_pool(name="xp", bufs=3) as xp,
            tc.tile_pool(name="fstate", bufs=1) as fstate,
            tc.tile_pool(name="gp", bufs=2) as gp,
            tc.tile_pool(name="psg", bufs=2, space="PSUM") as psg,
            tc.tile_pool(name="psf", bufs=2, space="PSUM") as psf,
        ):
            wx_sb = const.tile([P, 4, 2, 2, P], BF16)
            wu_sb = const.tile([P, 8, 2, 2, P], BF16)
            bs_sb = const.tile([P, 4, 2], F32)

            # chunk width per level: wide at the bulky bottom (fewer, cheaper
            # ACT/DMA instructions), narrow at the top (short serial tails at
            # the level boundaries, which are latency-bound)
            CW = {LTOP: 1024, LTOP - 1: 512, LTOP - 2: 512, LTOP - 3: 256,
                  LTOP - 4: 256}

            def level(lvl, h_prev, c_prev, on_chunk=None):
                n = N_L[lvl]
                cw = CW[lvl]
                leaf = lvl == LTOP
                h_cur = fstate.tile([P, 2, n], BF16, tag=f"h{lvl % 2}", name="h")
                c_cur = fstate.tile([P, 2, n], BF16, tag=f"c{lvl % 2}", name="c")
                half = N_L[lvl + 1] // 2 if not leaf else 0
                nch = (n + cw - 1) // cw
                # paired order: chunk j's parent needs child chunks (j, j+nch/2)
                order = []
                for j in range(nch // 2):
                    order += [j, nch // 2 + j]
                if not order:
                    order = list(range(nch))
                for ci in order:
                    s = ci * cw
                    ch = min(cw, n - s)
                    e = s + ch
                    nsb = (ch + 511) // 512  # 512-wide PSUM sub-banks
                    first = leaf and ci == 0
                    xt_t = xp.tile([P, 2, 1024], BF16, tag="x", name="x")
                    blk = (OFF[lvl] + s) // CHUNK
                    w0 = (OFF[lvl] + s) % CHUNK
                    if first:
                        # split so the first matmuls start after half the data
                        hw_ = ch // 2
                        nc.sync.dma_start(
                            xt_t[:, :, :hw_], xtb[blk][:, :, w0 : w0 + hw_]
                        )
                        nc.sync.dma_start(
                            xt_t[:, :, hw_:ch], xtb[blk][:, :, w0 + hw_ : w0 + ch]
                        )
                    else:
                        nc.sync.dma_start(xt_t[:, :, :ch], xtb[blk][:, :, w0 : w0 + ch])
                    if on_chunk is not None:
                        on_chunk(ci, c_cur)

                    g_i = gp.tile([P, 2, 1024], BF16, tag="gi", name="gi")
                    g_o = gp.tile([P, 2, 1024], BF16, tag="go", name="go")
                    g_u = gp.tile([P, 2, 1024], BF16, tag="gu", name="gu")
                    if not leaf:
                        fq = gp.tile([P, 2, 2, 1024], BF16, tag="fq", name="fq")
                        f_t = gp.tile([P, 2, 2, 1024], BF16, tag="ft", name="ft")

                    for mo in range(2):

                        def gate_mm(pt, g_idx, u0_idx, u1_idx):
                            # ko-outer so the two sub-banks share one
                            # LDWEIGHTS per weight (sb-outer on the very
                            # first chunk: its xt DMA arrives in halves)
                            nmm = 2 if leaf else 6
                            k = 0
                            if first and nsb == 2:
                                for sb in range(2):
                                    b0 = sb * 512
                                    for ko in range(2):
                                        nc.tensor.matmul(
                                            pt[:, b0 : b0 + 512],
                                            lhsT=wx_sb[:, g_idx, ko, mo],
                                            rhs=xt_t[:, ko, b0 : b0 + 512],
                                            start=(ko == 0),
                                            stop=(ko == nmm - 1),
                                        )
                                k = 2
                            else:
                                for ko in range(2):
                                    for sb in range(nsb):
                                        b0 = sb * 512
                                        w = min(512, ch - b0)
                                        nc.tensor.matmul(
                                            pt[:, b0 : b0 + w],
                                            lhsT=wx_sb[:, g_idx, ko, mo],
                                            rhs=xt_t[:, ko, b0 : b0 + w],
                                            start=(k == 0),
                                            stop=(k == nmm - 1),
                                        )
                                    k += 1
                            if not leaf:
                                for u_idx, base in ((u0_idx, s), (u1_idx, half + s)):
                                    for ko in range(2):
                                        for sb in range(nsb):
                                            b0 = sb * 512
                                            w = min(512, ch - b0)
                                            nc.tensor.matmul(
                                                pt[:, b0 : b0 + w],
                                                lhsT=wu_sb[:, u_idx, ko, mo],
                                                rhs=h_prev[
                                                    :, ko, base + b0 : base + b0 + w
                                                ],
                                                start=False,
                                                stop=(k == nmm - 1),
                                            )
                                        k += 1

                        def act_gate(dst, g_idx, func, u0_idx=0, u1_idx=0):
                            pt = psg.tile([P, 1024], F32, tag="ps", name="ps")[:, :ch]
                            gate_mm(pt, g_idx, u0_idx, u1_idx)
                            nc.scalar.activation(
                                dst[:, mo, :ch], pt, func,
                                bias=bs_sb[:, g_idx, mo : mo + 1],
                            )

                        act_gate(g_i, G_I, AF.Sigmoid, U_I0, U_I1)
                        act_gate(g_u, G_U, AF.Tanh, U_U0, U_U1)
                        act_gate(g_o, G_O, AF.Sigmoid, U_O0, U_O1)

                        if not leaf:
                            # zf = Wf x (no bias; bias folded into the STT).
                            # STT can read only one PSUM operand, so zf is
                            # staged to SBUF (bf16) with a DVE copy.
                            zf = psf.tile([P, 1024], F32, tag="pf", name="zf")[:, :ch]
                            for ko in range(2):
                                for sb in range(nsb):
                                    b0 = sb * 512
                                    w = min(512, ch - b0)
                                    nc.tensor.matmul(
                                        zf[:, b0 : b0 + w],
                                        lhsT=wx_sb[:, G_F, ko, mo],
                                        rhs=xt_t[:, ko, b0 : b0 + w],
                                        start=(ko == 0),
                                        stop=(ko == 1),
                                    )
                            zf_sb = gp.tile([P, 1024], BF16, tag="zf", name="zf_sb")
                            nc.vector.tensor_copy(zf_sb[:, :ch], zf)
                            for fi, u_idx, base in (
                                (0, U_F0, s),
                                (1, U_F1, half + s),
                            ):
                                pt = psf.tile([P, 1024], F32, tag="pf", name="pf")[:, :ch]
                                for ko in range(2):
                                    for sb in range(nsb):
                                        b0 = sb * 512
                                        w = min(512, ch - b0)
                                        nc.tensor.matmul(
                                            pt[:, b0 : b0 + w],
                                            lhsT=wu_sb[:, u_idx, ko, mo],
                                            rhs=h_prev[:, ko, base + b0 : base + b0 + w],
                                            start=(ko == 0),
                                            stop=(ko == 1),
                                        )
                                # fq = (Uf h + bf) + Wf x
                                nc.vector.scalar_tensor_tensor(
                                    out=fq[:, mo, fi, :ch],
                                    in0=pt,
                                    scalar=bs_sb[:, G_F, mo : mo + 1],
                                    in1=zf_sb[:, :ch],
                                    op0=ALU.add,
                                    op1=ALU.add,
                                )
                            # split sigmoid per mo so the t0/t1 muls can
                            # start as soon as their half is ready
                            nc.scalar.activation(
                                f_t[:, mo, :, :ch], fq[:, mo, :, :ch], AF.Sigmoid
                            )

                    # ---- elementwise (both mo at once, 2D APs, bf16 2x) ----
                    # top levels split the chain in half-chunks so the next
                    # level's matmuls start after half the tail latency
                    th = gp.tile([P, 2, 1024], BF16, tag="th", name="th")
                    if leaf:
                        c_ap = c_cur[:, :, s:e]
                        nc.vector.tensor_mul(
                            out=c_ap, in0=g_i[:, :, :ch], in1=g_u[:, :, :ch]
                        )
                        nc.scalar.activation(th[:, :, :ch], c_ap, AF.Tanh)
                        nc.vector.tensor_mul(
                            out=h_cur[:, :, s:e], in0=g_o[:, :, :ch], in1=th[:, :, :ch]
                        )
                    else:
                        iu = gp.tile([P, 2, 1024], BF16, tag="iu", name="iu")
                        nc.vector.tensor_mul(
                            out=iu[:, :, :ch], in0=g_i[:, :, :ch], in1=g_u[:, :, :ch]
                        )
                        t0 = gp.tile([P, 2, 1024], BF16, tag="t0", name="t0")
                        t1 = gp.tile([P, 2, 1024], BF16, tag="t1", name="t1")
                        nhs = 2 if lvl <= LTOP - 2 else 1
                        hw2 = ch // nhs
                        for hs in range(nhs):
                            b0 = hs * hw2
                            b1 = b0 + hw2
                            c_ap = c_cur[:, :, s + b0 : s + b1]
                            nc.vector.tensor_mul(
                                out=t0[:, :, b0:b1],
                                in0=f_t[:, :, 0, b0:b1],
                                in1=c_prev[:, :, s + b0 : s + b1],
                            )
                            nc.vector.tensor_mul(
                                out=t1[:, :, b0:b1],
                                in0=f_t[:, :, 1, b0:b1],
                                in1=c_prev[:, :, half + s + b0 : half + s + b1],
                            )
                            nc.vector.tensor_add(
                                out=c_ap, in0=iu[:, :, b0:b1], in1=t0[:, :, b0:b1]
                            )
                            nc.vector.tensor_add(
                                out=c_ap, in0=c_ap, in1=t1[:, :, b0:b1]
                            )
                            nc.scalar.activation(th[:, :, b0:b1], c_ap, AF.Tanh)
                            nc.vector.tensor_mul(
                                out=h_cur[:, :, s + b0 : s + b1],
                                in0=g_o[:, :, b0:b1],
                                in1=th[:, :, b0:b1],
                            )
                return h_cur, c_cur

            def on_chunk_leaf(ci, c_cur):
                # The U-table (and Wf) DMA is gated behind the second
                # processed leaf chunk (WAW via a 1-elem copy that reads leaf
                # c) so its HBM traffic cannot starve the leaf xt stream.
                if ci == 2:
                    nc.vector.tensor_copy(
                        wu_sb[0:1, 0, 0, 0, 0:1], c_cur[0:1, 0, 0:1]
                    )
                    nc.sync.dma_start(wu_sb[:], wu)
                    nc.sync.dma_start(wx_sb[:, 3:4], wx[:, 3:4])

            # x-weights + biases first: the first matmul needs them
            nc.sync.dma_start(wx_sb[:, 0:3], wx[:, 0:3])
            nc.sync.dma_start(bs_sb[:], bs)

            h_prev = c_prev = None
            for lvl in range(LTOP, CUT - 1, -1):
                h_prev, c_prev = level(
                    lvl, h_prev, c_prev,
                    on_chunk=on_chunk_leaf if lvl == LTOP else None,
                )

            nc.sync.dma_start(hc[:, 0], h_prev)
            nc.sync.dma_start(hc[:, 1], c_prev)
    nc.compile()
    return nc


_CACHE = {}


def _get_program():
    if "nc" not in _CACHE:
        _CACHE["nc"] = _build_program()
    return _CACHE["nc"]


def _core_index_table():
    if "idx" in _CACHE:
        return _CACHE["idx"]
    idx = np.zeros((NCORES, NSLOT), dtype=np.int64)
    for lvl in range(LTOP, CUT - 1, -1):
        n = N_L[lvl]
        rev = _bitrev(lvl - 3)
        start = (1 << lvl) - 1
        for m in range(NCORES):
            pos = m * n + rev
            idx[m, OFF[lvl] : OFF[lvl] + n] = start + pos
    _CACHE["idx"] = idx
    return idx


def _pack_w(mat):
    """[out,in] (256,256) -> [p, ko, mo, m] = W.T[ko*128+p, mo*128+m]."""
    return mat.reshape(2, P, 2, P).transpose(3, 2, 0, 1)


def _sigmoid(x):
    return 1.0 / (1.0 + np.exp(-x))


def _host_node_batch(x, ch_h, ch_c, prm):
    (Wi, bi, Ui, Wo, bo, Uo, Wu, bu, Uu, Wf, bf, Uf) = prm

    def gate(W, b, U):
        return x @ W.T + b + ch_h[:, 0] @ U[0].T + ch_h[:, 1] @ U[1].T

    i = _sigmoid(gate(Wi, bi, Ui))
    o = _sigmoid(gate(Wo, bo, Uo))
    u = np.tanh(gate(Wu, bu, Uu))
    xf = x @ Wf.T + bf
    f0 = _sigmoid(xf + ch_h[:, 0] @ Uf[0].T)
    f1 = _sigmoid(xf + ch_h[:, 1] @ Uf[1].T)
    c = i * u + f0 * ch_c[:, 0] + f1 * ch_c[:, 1]
    h = o * np.tanh(c)
    return h.astype(np.float32), c.astype(np.float32)


def kernel(emb, W_i, b_i, U_i, W_o, b_o, U_o, W_u, b_u, U_u, W_f, b_f, U_f):
    emb = np.asarray(emb, dtype=np.float32)
    f = lambda a: np.asarray(a, dtype=np.float32)
    W_i, b_i, U_i = f(W_i), f(b_i), f(U_i)
    W_o, b_o, U_o = f(W_o), f(b_o), f(U_o)
    W_u, b_u, U_u = f(W_u), f(b_u), f(U_u)
    W_f, b_f, U_f = f(W_f), f(b_f), f(U_f)

    nc = _get_program()
    idx = _core_index_table()

    wx = np.ascontiguousarray(
        np.stack([_pack_w(m) for m in (W_i, W_o, W_u, W_f)], axis=1)
    ).astype(NPBF16)
    wu = np.ascontiguousarray(
        np.stack(
            [
                _pack_w(m)
                for m in (
                    U_i[0], U_i[1], U_o[0], U_o[1],
                    U_u[0], U_u[1], U_f[0], U_f[1],
                )
            ],
            axis=1,
        )
    ).astype(NPBF16)
    bsv = np.ascontiguousarray(
        np.stack([b.reshape(2, P).T for b in (b_i, b_o, b_u, b_f)], axis=1)
    )

    in_maps = []
    npad = NBLK * CHUNK
    for m in range(NCORES):
        xm = emb[idx[m]]  # [NSLOT, 256]
        arr = np.zeros((256, npad), dtype=np.float32)
        arr[:, :NSLOT] = xm.T
        xtc = np.ascontiguousarray(
            arr.reshape(2, P, NBLK, CHUNK).transpose(2, 1, 0, 3)
        ).astype(NPBF16)  # [blk, p, ko, s]
        in_maps.append({"xtb": xtc, "wx": wx, "wu": wu, "bs": bsv})

    kw = {}
    if os.environ.get("KERNEL_TRACE_DIR"):
        kw = {"trace": True, "tmpdir": os.environ["KERNEL_TRACE_DIR"]}
    res = run_bass_kernel_spmd(nc, in_maps, core_ids=list(range(NCORES)), **kw)
    _CACHE["last_res"] = res

    rev = _bitrev(CUT - 3)
    n_cut = 1 << CUT
    h = np.zeros((n_cut, H), dtype=np.float32)
    c = np.zeros((n_cut, H), dtype=np.float32)
    for m in range(NCORES):
        out = np.asarray(res.results[m]["hc"]).astype(np.float32)  # [P,2,2,NOUT]
        pos = m * NOUT + rev
        # feature-major [p, hc, ko, j] -> node-major [j, ko*128+p]
        h[pos] = out[:, 0].transpose(1, 0, 2).reshape(H, NOUT).T
        c[pos] = out[:, 1].transpose(1, 0, 2).reshape(H, NOUT).T

    prm = (W_i, b_i, U_i, W_o, b_o, U_o, W_u, b_u, U_u, W_f, b_f, U_f)
    for lvl in range(CUT - 1, -1, -1):
        n = 1 << lvl
        start = n - 1
        ch_h = h.reshape(n, 2, H)
        ch_c = c.reshape(n, 2, H)
        h, c = _host_node_batch(emb[start : start + n], ch_h, ch_c, prm)

    return h[0], c[0]


# revision 13
# speedup vs baseline: 1.0797x; 1.0797x over previous
"""Bass/Trainium2 kernel for nn_NaryTreeLSTM (binary TreeLSTM over a complete
depth-16 tree, H=D=256, heap/level node order).

Sharding: data-parallel over 8 independent subtrees (core m owns the subtree
rooted at level-3 node m). The device computes levels 15..CUT per core in a
single F-layout (feature-on-partition, nodes-on-free) pipeline; the tiny top
of the tree (2^CUT-1 nodes) is finished on host during the gather/unshard
step (the cross-core combine has to leave the device at level 3 anyway, and
the top levels are latency-bound serial remnants on device).

v2 vs v1: all-bf16 matmul/elementwise datapath (error ~4e-3, well under the
2e-2 gate; halves HBM traffic and enables fast weight loads), Wf*x computed
once per node and shared between both forget gates via a fused
scalar_tensor_tensor on DVE, wide multi-bank ACTIVATEs (the scalar engine
has no instruction pipelining, so per-instruction fixed cost ~290ns
dominates), and both output-feature halves processed per chunk with single
2D-AP DVE ops in 2x bf16 mode. The N-layout tail + PE transposes of v1 are
gone entirely (CUT=11).

Node order per level is bit-reversed so the even/odd children of a
contiguous parent chunk are the first/second half of the child level.

Per node (children h_e,h_o / c_e,c_o; x = emb row):
  i = sig(Wi x + bi + Ui0 h_e + Ui1 h_o)      o, u analogous (u: tanh)
  f0 = sig(Wf x + bf + Uf0 h_e),  f1 = sig(Wf x + bf + Uf1 h_o)
  c = i*u + f0*c_e + f1*c_o ;  h = o * tanh(c)
"""

import os

import numpy as np

try:
    import concourse  # noqa: F401
except ImportError:  # pragma: no cover
    import sys

    sys.path.insert(0, "/opt/trn_rl_repo")

import ml_dtypes

import concourse.tile as tile
from concourse import bacc, mybir
from concourse.bass_utils import run_bass_kernel_spmd

F32 = mybir.dt.float32
BF16 = mybir.dt.bfloat16
AF = mybir.ActivationFunctionType
ALU = mybir.AluOpType
NPBF16 = ml_dtypes.bfloat16

DEPTH = 16
H = 256
P = 128
NCORES = 8
LTOP = DEPTH - 1
CUT = int(os.environ.get("TREELSTM_CUT", "11"))  # device: levels 15..CUT
CHUNK = 1024  # two PSUM banks per gate tile; one ACTIVATE per gate per chunk

N_L = {l: 1 << (l - 3) for l in range(CUT, LTOP + 1)}
NSLOT = sum(N_L.values())
OFF = {}
_o = 0
for _l in range(LTOP, CUT - 1, -1):
    OFF[_l] = _o
    _o += N_L[_l]
NOUT = N_L[CUT]
NBLK = (NSLOT + CHUNK - 1) // CHUNK

# weight table gate indices
G_I, G_O, G_U, G_F = 0, 1, 2, 3
U_I0, U_I1, U_O0, U_O1, U_U0, U_U1, U_F0, U_F1 = range(8)


def _bitrev(nbits):
    n = 1 << nbits
    r = np.zeros(n, dtype=np.int64)
    for j in range(n):
        v = 0
        for b in range(nbits):
            if j & (1 << b):
                v |= 1 << (nbits - 1 - b)
        r[j] = v
    return r


def _build_program():
    nc = bacc.Bacc("TRN2", target_bir_lowering=False, debug=False, num_devices=NCORES)
    xtb = nc.dram_tensor("xtb", [NBLK, P, 2, CHUNK], BF16, kind="ExternalInput").ap()
    wx = nc.dram_tensor("wx", [P, 4, 2, 2, P], BF16, kind="ExternalInput").ap()
    wu = nc.dram_tensor("wu", [P, 8, 2, 2, P], BF16, kind="ExternalInput").ap()
    bs = nc.dram_tensor("bs", [P, 4, 2], F32, kind="ExternalInput").ap()
    hc = nc.dram_tensor("hc", [P, 2, 2, NOUT], BF16, kind="ExternalOutput").ap()

    with tile.TileContext(nc) as tc:
        with (
            tc.tile_pool(name="const", bufs=1) as const,
            tc.tile_pool(name="xp", bufs=3) as xp,
            tc.tile_pool(name="fstate", bufs=1) as fstate,
            tc.tile_pool(name="gp", bufs=2) as gp,
            tc.tile_pool(name="psg", bufs=2, space="PSUM") as psg,
            tc.tile_pool(name="psf", bufs=2, space="PSUM") as psf,
        ):
            wx_sb = const.tile([P, 4, 2, 2, P], BF16)
            wu_sb = const.tile([P, 8, 2, 2, P], BF16)
            bs_sb = const.tile([P, 4, 2], F32)

            # chunk width per level: wide at the bulky bottom (fewer, cheaper
            # ACT/DMA instructions), narrow at the top (short serial tails at
            # the level boundaries, which are latency-bound)
            CW = {LTOP: 1024, LTOP - 1: 512, LTOP - 2: 512, LTOP - 3: 256,
                  LTOP - 4: 256}

            def level(lvl, h_prev, c_prev, on_chunk=None):
                n = N_L[lvl]
                cw = CW[lvl]
                leaf = lvl == LTOP
                h_cur = fstate.tile([P, 2, n], BF16, tag=f"h{lvl % 2}", name="h")
                c_cur = fstate.tile([P, 2, n], BF16, tag=f"c{lvl % 2}", name="c")
                half = N_L[lvl + 1] // 2 if not leaf else 0
                nch = (n + cw - 1) // cw
                # paired order: chunk j's parent needs child chunks (j, j+nch/2)
                order = []
                for j in range(nch // 2):
                    order += [j, nch // 2 + j]
                if not order:
                    order = list(range(nch))
                for ci in order:
                    s = ci * cw
                    ch = min(cw, n - s)
                    e = s + ch
                    nsb = (ch + 511) // 512  # 512-wide PSUM sub-banks
                    first = False
                    xt_t = xp.tile([P, 2, 1024], BF16, tag="x", name="x")
                    blk = (OFF[lvl] + s) // CHUNK
                    w0 = (OFF[lvl] + s) % CHUNK
                    nc.sync.dma_start(xt_t[:, :, :ch], xtb[blk][:, :, w0 : w0 + ch])
                    if on_chunk is not None:
                        on_chunk(ci, c_cur)

                    g_i = gp.tile([P, 2, 1024], BF16, tag="gi", name="gi")
                    g_o = gp.tile([P, 2, 1024], BF16, tag="go", name="go")
                    g_u = gp.tile([P, 2, 1024], BF16, tag="gu", name="gu")
                    if not leaf:
                        fq = gp.tile([P, 2, 2, 1024], BF16, tag="fq", name="fq")
                        f_t = gp.tile([P, 2, 2, 1024], BF16, tag="ft", name="ft")

                    for mo in range(2):

                        def gate_mm(pt, g_idx, u0_idx, u1_idx):
                            # ko-outer so the two sub-banks share one
                            # LDWEIGHTS per weight (sb-outer on the very
                            # first chunk: its xt DMA arrives in halves)
                            nmm = 2 if leaf else 6
                            k = 0
                            if first and nsb == 2:
                                for sb in range(2):
                                    b0 = sb * 512
                                    for ko in range(2):
                                        nc.tensor.matmul(
                                            pt[:, b0 : b0 + 512],
                                            lhsT=wx_sb[:, g_idx, ko, mo],
                                            rhs=xt_t[:, ko, b0 : b0 + 512],
                                            start=(ko == 0),
                                            stop=(ko == nmm - 1),
                                        )
                                k = 2
                            else:
                                for ko in range(2):
                                    for sb in range(nsb):
                                        b0 = sb * 512
                                        w = min(512, ch - b0)
                                        nc.tensor.matmul(
                                            pt[:, b0 : b0 + w],
                                            lhsT=wx_sb[:, g_idx, ko, mo],
                                            rhs=xt_t[:, ko, b0 : b0 + w],
                                            start=(k == 0),
                                            stop=(k == nmm - 1),
                                        )
                                    k += 1
                            if not leaf:
                                for u_idx, base in ((u0_idx, s), (u1_idx, half + s)):
                                    for ko in range(2):
                                        for sb in range(nsb):
                                            b0 = sb * 512
                                            w = min(512, ch - b0)
                                            nc.tensor.matmul(
                                                pt[:, b0 : b0 + w],
                                                lhsT=wu_sb[:, u_idx, ko, mo],
                                                rhs=h_prev[
                                                    :, ko, base + b0 : base + b0 + w
                                                ],
                                                start=False,
                                                stop=(k == nmm - 1),
                                            )
                                        k += 1

                        def act_gate(dst, g_idx, func, u0_idx=0, u1_idx=0):
                            pt = psg.tile([P, 1024], F32, tag="ps", name="ps")[:, :ch]
                            gate_mm(pt, g_idx, u0_idx, u1_idx)
                            nc.scalar.activation(
                                dst[:, mo, :ch], pt, func,
                                bias=bs_sb[:, g_idx, mo : mo + 1],
                            )

                        act_gate(g_i, G_I, AF.Sigmoid, U_I0, U_I1)
                        act_gate(g_u, G_U, AF.Tanh, U_U0, U_U1)
                        act_gate(g_o, G_O, AF.Sigmoid, U_O0, U_O1)

                        if not leaf:
                            # zf = Wf x (no bias; bias folded into the STT).
                            # STT can read only one PSUM operand, so zf is
                            # staged to SBUF (bf16) with a DVE copy.
                            zf = psf.tile([P, 1024], F32, tag="pf", name="zf")[:, :ch]
                            for ko in range(2):
                                for sb in range(nsb):
                                    b0 = sb * 512
                                    w = min(512, ch - b0)
                                    nc.tensor.matmul(
                                        zf[:, b0 : b0 + w],
                                        lhsT=wx_sb[:, G_F, ko, mo],
                                        rhs=xt_t[:, ko, b0 : b0 + w],
                                        start=(ko == 0),
                                        stop=(ko == 1),
                                    )
                            zf_sb = gp.tile([P, 1024], BF16, tag="zf", name="zf_sb")
                            nc.vector.tensor_copy(zf_sb[:, :ch], zf)
                            for fi, u_idx, base in (
                                (0, U_F0, s),
                                (1, U_F1, half + s),
                            ):
                                pt = psf.tile([P, 1024], F32, tag="pf", name="pf")[:, :ch]
                                for ko in range(2):
                                    for sb in range(nsb):
                                        b0 = sb * 512
                                        w = min(512, ch - b0)
                                        nc.tensor.matmul(
                                            pt[:, b0 : b0 + w],
                                            lhsT=wu_sb[:, u_idx, ko, mo],
                                            rhs=h_prev[:, ko, base + b0 : base + b0 + w],
                                            start=(ko == 0),
                                            stop=(ko == 1),
                                        )
                                # fq = (Uf h + bf) + Wf x
                                nc.vector.scalar_tensor_tensor(
                                    out=fq[:, mo, fi, :ch],
                                    in0=pt,
                                    scalar=bs_sb[:, G_F, mo : mo + 1],
                                    in1=zf_sb[:, :ch],
                                    op0=ALU.add,
                                    op1=ALU.add,
                                )
                            # split sigmoid per mo so the t0/t1 muls can
                            # start as soon as their half is ready
                            nc.scalar.activation(
                                f_t[:, mo, :, :ch], fq[:, mo, :, :ch], AF.Sigmoid
                            )

                    # ---- elementwise (both mo at once, 2D APs, bf16 2x) ----
                    # top levels split the chain in half-chunks so the next
                    # level's matmuls start after half the tail latency
                    th = gp.tile([P, 2, 1024], BF16, tag="th", name="th")
                    if leaf:
                        c_ap = c_cur[:, :, s:e]
                        nc.vector.tensor_mul(
                            out=c_ap, in0=g_i[:, :, :ch], in1=g_u[:, :, :ch]
                        )
                        nc.scalar.activation(th[:, :, :ch], c_ap, AF.Tanh)
                        nc.vector.tensor_mul(
                            out=h_cur[:, :, s:e], in0=g_o[:, :, :ch], in1=th[:, :, :ch]
                        )
                    else:
                        iu = gp.tile([P, 2, 1024], BF16, tag="iu", name="iu")
                        nc.vector.tensor_mul(
                            out=iu[:, :, :ch], in0=g_i[:, :, :ch], in1=g_u[:, :, :ch]
                        )
                        t0 = gp.tile([P, 2, 1024], BF16, tag="t0", name="t0")
                        t1 = gp.tile([P, 2, 1024], BF16, tag="t1", name="t1")
                        nhs = 2 if lvl <= LTOP - 2 else 1
                        hw2 = ch // nhs
                        for hs in range(nhs):
                            b0 = hs * hw2
                            b1 = b0 + hw2
                            c_ap = c_cur[:, :, s + b0 : s + b1]
                            nc.vector.tensor_mul(
                                out=t0[:, :, b0:b1],
                                in0=f_t[:, :, 0, b0:b1],
                                in1=c_prev[:, :, s + b0 : s + b1],
                            )
                            nc.vector.tensor_mul(
                                out=t1[:, :, b0:b1],
                                in0=f_t[:, :, 1, b0:b1],
                                in1=c_prev[:, :, half + s + b0 : half + s + b1],
                            )
                            nc.vector.tensor_add(
                                out=c_ap, in0=iu[:, :, b0:b1], in1=t0[:, :, b0:b1]
                            )
                            nc.vector.tensor_add(
                                out=c_ap, in0=c_ap, in1=t1[:, :, b0:b1]
                            )
                            nc.scalar.activation(th[:, :, b0:b1], c_ap, AF.Tanh)
                            nc.vector.tensor_mul(
                                out=h_cur[:, :, s + b0 : s + b1],
                                in0=g_o[:, :, b0:b1],
                                in1=th[:, :, b0:b1],
                            )
                return h_cur, c_cur

            def on_chunk_leaf(ci, c_cur):
                # The U-table (and Wf) DMA is gated behind the second
                # processed leaf chunk (WAW via a 1-elem copy that reads leaf
                # c) so its HBM traffic cannot starve the leaf xt stream.
                if ci == 2:
                    nc.vector.tensor_copy(
                        wu_sb[0:1, 0, 0, 0, 0:1], c_cur[0:1, 0, 0:1]
                    )
                    nc.sync.dma_start(wu_sb[:], wu)
                    nc.sync.dma_start(wx_sb[:, 3:4], wx[:, 3:4])

            # x-weights + biases first: the first matmul needs them
            nc.sync.dma_start(wx_sb[:, 0:3], wx[:, 0:3])
            nc.sync.dma_start(bs_sb[:], bs)

            h_prev = c_prev = None
            for lvl in range(LTOP, CUT - 1, -1):
                h_prev, c_prev = level(
                    lvl, h_prev, c_prev,
                    on_chunk=on_chunk_leaf if lvl == LTOP else None,
                )

            nc.sync.dma_start(hc[:, 0], h_prev)
            nc.sync.dma_start(hc[:, 1], c_prev)
    nc.compile()
    return nc


_CACHE = {}


def _get_program():
    if "nc" not in _CACHE:
        _CACHE["nc"] = _build_program()
    return _CACHE["nc"]


def _core_index_table():
    if "idx" in _CACHE:
        return _CACHE["idx"]
    idx = np.zeros((NCORES, NSLOT), dtype=np.int64)
    for lvl in range(LTOP, CUT - 1, -1):
        n = N_L[lvl]
        rev = _bitrev(lvl - 3)
        start = (1 << lvl) - 1
        for m in range(NCORES):
            pos = m * n + rev
            idx[m, OFF[lvl] : OFF[lvl] + n] = start + pos
    _CACHE["idx"] = idx
    return idx


def _pack_w(mat):
    """[out,in] (256,256) -> [p, ko, mo, m] = W.T[ko*128+p, mo*128+m]."""
    return mat.reshape(2, P, 2, P).transpose(3, 2, 0, 1)


def _sigmoid(x):
    return 1.0 / (1.0 + np.exp(-x))


def _host_node_batch(x, ch_h, ch_c, prm):
    (Wi, bi, Ui, Wo, bo, Uo, Wu, bu, Uu, Wf, bf, Uf) = prm

    def gate(W, b, U):
        return x @ W.T + b + ch_h[:, 0] @ U[0].T + ch_h[:, 1] @ U[1].T

    i = _sigmoid(gate(Wi, bi, Ui))
    o = _sigmoid(gate(Wo, bo, Uo))
    u = np.tanh(gate(Wu, bu, Uu))
    xf = x @ Wf.T + bf
    f0 = _sigmoid(xf + ch_h[:, 0] @ Uf[0].T)
    f1 = _sigmoid(xf + ch_h[:, 1] @ Uf[1].T)
    c = i * u + f0 * ch_c[:, 0] + f1 * ch_c[:, 1]
    h = o * np.tanh(c)
    return h.astype(np.float32), c.astype(np.float32)


def kernel(emb, W_i, b_i, U_i, W_o, b_o, U_o, W_u, b_u, U_u, W_f, b_f, U_f):
    emb = np.asarray(emb, dtype=np.float32)
    f = lambda a: np.asarray(a, dtype=np.float32)
    W_i, b_i, U_i = f(W_i), f(b_i), f(U_i)
    W_o, b_o, U_o = f(W_o), f(b_o), f(U_o)
    W_u, b_u, U_u = f(W_u), f(b_u), f(U_u)
    W_f, b_f, U_f = f(W_f), f(b_f), f(U_f)

    nc = _get_program()
    idx = _core_index_table()

    wx = np.ascontiguousarray(
        np.stack([_pack_w(m) for m in (W_i, W_o, W_u, W_f)], axis=1)
    ).astype(NPBF16)
    wu = np.ascontiguousarray(
        np.stack(
            [
                _pack_w(m)
                for m in (
                    U_i[0], U_i[1], U_o[0], U_o[1],
                    U_u[0], U_u[1], U_f[0], U_f[1],
                )
            ],
            axis=1,
        )
    ).astype(NPBF16)
    bsv = np.ascontiguousarray(
        np.stack([b.reshape(2, P).T for b in (b_i, b_o, b_u, b_f)], axis=1)
    )

    in_maps = []
    npad = NBLK * CHUNK
    for m in range(NCORES):
        xm = emb[idx[m]]  # [NSLOT, 256]
        arr = np.zeros((256, npad), dtype=np.float32)
        arr[:, :NSLOT] = xm.T
        xtc = np.ascontiguousarray(
            arr.reshape(2, P, NBLK, CHUNK).transpose(2, 1, 0, 3)
        ).astype(NPBF16)  # [blk, p, ko, s]
        in_maps.append({"xtb": xtc, "wx": wx, "wu": wu, "bs": bsv})

    kw = {}
    if os.environ.get("KERNEL_TRACE_DIR"):
        kw = {"trace": True, "tmpdir": os.environ["KERNEL_TRACE_DIR"]}
    res = run_bass_kernel_spmd(nc, in_maps, core_ids=list(range(NCORES)), **kw)
    _CACHE["last_res"] = res

    rev = _bitrev(CUT - 3)
    n_cut = 1 << CUT
    h = np.zeros((n_cut, H), dtype=np.float32)
    c = np.zeros((n_cut, H), dtype=np.float32)
    for m in range(NCORES):
        out = np.asarray(res.results[m]["hc"]).astype(np.float32)  # [P,2,2,NOUT]
        pos = m * NOUT + rev
        # feature-major [p, hc, ko, j] -> node-major [j, ko*128+p]
        h[pos] = out[:, 0].transpose(1, 0, 2).reshape(H, NOUT).T
        c[pos] = out[:, 1].transpose(1, 0, 2).reshape(H, NOUT).T

    prm = (W_i, b_i, U_i, W_o, b_o, U_o, W_u, b_u, U_u, W_f, b_f, U_f)
    for lvl in range(CUT - 1, -1, -1):
        n = 1 << lvl
        start = n - 1
        ch_h = h.reshape(n, 2, H)
        ch_c = c.reshape(n, 2, H)
        h, c = _host_node_batch(emb[start : start + n], ch_h, ch_c, prm)

    return h[0], c[0]


# revision 15
# speedup vs baseline: 1.0876x; 1.0073x over previous
"""Bass/Trainium2 kernel for nn_NaryTreeLSTM (binary TreeLSTM over a complete
depth-16 tree, H=D=256, heap/level node order).

Sharding: data-parallel over 8 independent subtrees (core m owns the subtree
rooted at level-3 node m). The device computes levels 15..CUT per core in a
single F-layout (feature-on-partition, nodes-on-free) pipeline; the tiny top
of the tree (2^CUT-1 nodes) is finished on host during the gather/unshard
step (the cross-core combine has to leave the device at level 3 anyway, and
the top levels are latency-bound serial remnants on device).

v2 vs v1: all-bf16 matmul/elementwise datapath (error ~4e-3, well under the
2e-2 gate; halves HBM traffic and enables fast weight loads), Wf*x computed
once per node and shared between both forget gates via a fused
scalar_tensor_tensor on DVE, wide multi-bank ACTIVATEs (the scalar engine
has no instruction pipelining, so per-instruction fixed cost ~290ns
dominates), and both output-feature halves processed per chunk with single
2D-AP DVE ops in 2x bf16 mode. The N-layout tail + PE transposes of v1 are
gone entirely (CUT=11).

Node order per level is bit-reversed so the even/odd children of a
contiguous parent chunk are the first/second half of the child level.

Per node (children h_e,h_o / c_e,c_o; x = emb row):
  i = sig(Wi x + bi + Ui0 h_e + Ui1 h_o)      o, u analogous (u: tanh)
  f0 = sig(Wf x + bf + Uf0 h_e),  f1 = sig(Wf x + bf + Uf1 h_o)
  c = i*u + f0*c_e + f1*c_o ;  h = o * tanh(c)
"""

import os

import numpy as np

try:
    import concourse  # noqa: F401
except ImportError:  # pragma: no cover
    import sys

    sys.path.insert(0, "/opt/trn_rl_repo")

import ml_dtypes

import concourse.tile as tile
from concourse import bacc, mybir
from concourse.bass_utils import run_bass_kernel_spmd

F32 = mybir.dt.float32
BF16 = mybir.dt.bfloat16
AF = mybir.ActivationFunctionType
ALU = mybir.AluOpType
NPBF16 = ml_dtypes.bfloat16

DEPTH = 16
H = 256
P = 128
NCORES = 8
LTOP = DEPTH - 1
CUT = int(os.environ.get("TREELSTM_CUT", "12"))  # device: levels 15..CUT
CHUNK = 1024  # two PSUM banks per gate tile; one ACTIVATE per gate per chunk

N_L = {l: 1 << (l - 3) for l in range(CUT, LTOP + 1)}
NSLOT = sum(N_L.values())
OFF = {}
_o = 0
for _l in range(LTOP, CUT - 1, -1):
    OFF[_l] = _o
    _o += N_L[_l]
NOUT = N_L[CUT]
NBLK = (NSLOT + CHUNK - 1) // CHUNK

# weight table gate indices
G_I, G_O, G_U, G_F = 0, 1, 2, 3
U_I0, U_I1, U_O0, U_O1, U_U0, U_U1, U_F0, U_F1 = range(8)


def _bitrev(nbits):
    n = 1 << nbits
    r = np.zeros(n, dtype=np.int64)
    for j in range(n):
        v = 0
        for b in range(nbits):
            if j & (1 << b):
                v |= 1 << (nbits - 1 - b)
        r[j] = v
    return r


def _build_program():
    nc = bacc.Bacc("TRN2", target_bir_lowering=False, debug=False, num_devices=NCORES)
    xtb = nc.dram_tensor("xtb", [NBLK, P, 2, CHUNK], BF16, kind="ExternalInput").ap()
    wx = nc.dram_tensor("wx", [P, 4, 2, 2, P], BF16, kind="ExternalInput").ap()
    wu = nc.dram_tensor("wu", [P, 8, 2, 2, P], BF16, kind="ExternalInput").ap()
    bs = nc.dram_tensor("bs", [P, 4, 2], F32, kind="ExternalInput").ap()
    hc = nc.dram_tensor("hc", [P, 2, 2, NOUT], BF16, kind="ExternalOutput").ap()

    with tile.TileContext(nc) as tc:
        with (
            tc.tile_pool(name="const", bufs=1) as const,
            tc.tile_pool(name="xp", bufs=3) as xp,
            tc.tile_pool(name="fstate", bufs=1) as fstate,
            tc.tile_pool(name="gp", bufs=2) as gp,
            tc.tile_pool(name="psg", bufs=2, space="PSUM") as psg,
            tc.tile_pool(name="psf", bufs=2, space="PSUM") as psf,
        ):
            wx_sb = const.tile([P, 4, 2, 2, P], BF16)
            wu_sb = const.tile([P, 8, 2, 2, P], BF16)
            bs_sb = const.tile([P, 4, 2], F32)

            # chunk width per level: wide at the bulky bottom (fewer, cheaper
            # ACT/DMA instructions), narrow at the top (short serial tails at
            # the level boundaries, which are latency-bound)
            CW = {LTOP: 1024, LTOP - 1: 512, LTOP - 2: 512, LTOP - 3: 256,
                  LTOP - 4: 256}

            def level(lvl, h_prev, c_prev, on_chunk=None):
                n = N_L[lvl]
                cw = CW[lvl]
                leaf = lvl == LTOP
                h_cur = fstate.tile([P, 2, n], BF16, tag=f"h{lvl % 2}", name="h")
                c_cur = fstate.tile([P, 2, n], BF16, tag=f"c{lvl % 2}", name="c")
                half = N_L[lvl + 1] // 2 if not leaf else 0
                nch = (n + cw - 1) // cw
                # paired order: chunk j's parent needs child chunks (j, j+nch/2)
                order = []
                for j in range(nch // 2):
                    order += [j, nch // 2 + j]
                if not order:
                    order = list(range(nch))
                for ci in order:
                    s = ci * cw
                    ch = min(cw, n - s)
                    e = s + ch
                    nsb = (ch + 511) // 512  # 512-wide PSUM sub-banks
                    first = False
                    xt_t = xp.tile([P, 2, 1024], BF16, tag="x", name="x")
                    blk = (OFF[lvl] + s) // CHUNK
                    w0 = (OFF[lvl] + s) % CHUNK
                    nc.sync.dma_start(xt_t[:, :, :ch], xtb[blk][:, :, w0 : w0 + ch])
                    if on_chunk is not None:
                        on_chunk(ci, c_cur)

                    g_i = gp.tile([P, 2, 1024], BF16, tag="gi", name="gi")
                    g_o = gp.tile([P, 2, 1024], BF16, tag="go", name="go")
                    g_u = gp.tile([P, 2, 1024], BF16, tag="gu", name="gu")
                    if not leaf:
                        fq = gp.tile([P, 2, 2, 1024], BF16, tag="fq", name="fq")
                        f_t = gp.tile([P, 2, 2, 1024], BF16, tag="ft", name="ft")

                    for mo in range(2):

                        def gate_mm(pt, g_idx, u0_idx, u1_idx):
                            # ko-outer so the two sub-banks share one
                            # LDWEIGHTS per weight (sb-outer on the very
                            # first chunk: its xt DMA arrives in halves)
                            nmm = 2 if leaf else 6
                            k = 0
                            if first and nsb == 2:
                                for sb in range(2):
                                    b0 = sb * 512
                                    for ko in range(2):
                                        nc.tensor.matmul(
                                            pt[:, b0 : b0 + 512],
                                            lhsT=wx_sb[:, g_idx, ko, mo],
                                            rhs=xt_t[:, ko, b0 : b0 + 512],
                                            start=(ko == 0),
                                            stop=(ko == nmm - 1),
                                        )
                                k = 2
                            else:
                                for ko in range(2):
                                    for sb in range(nsb):
                                        b0 = sb * 512
                                        w = min(512, ch - b0)
                                        nc.tensor.matmul(
                                            pt[:, b0 : b0 + w],
                                            lhsT=wx_sb[:, g_idx, ko, mo],
                                            rhs=xt_t[:, ko, b0 : b0 + w],
                                            start=(k == 0),
                                            stop=(k == nmm - 1),
                                        )
                                    k += 1
                            if not leaf:
                                for u_idx, base in ((u0_idx, s), (u1_idx, half + s)):
                                    for ko in range(2):
                                        for sb in range(nsb):
                                            b0 = sb * 512
                                            w = min(512, ch - b0)
                                            nc.tensor.matmul(
                                                pt[:, b0 : b0 + w],
                                                lhsT=wu_sb[:, u_idx, ko, mo],
                                                rhs=h_prev[
                                                    :, ko, base + b0 : base + b0 + w
                                                ],
                                                start=False,
                                                stop=(k == nmm - 1),
                                            )
                                        k += 1

                        def act_gate(dst, g_idx, func, u0_idx=0, u1_idx=0):
                            pt = psg.tile([P, 1024], F32, tag="ps", name="ps")[:, :ch]
                            gate_mm(pt, g_idx, u0_idx, u1_idx)
                            nc.scalar.activation(
                                dst[:, mo, :ch], pt, func,
                                bias=bs_sb[:, g_idx, mo : mo + 1],
                            )

                        act_gate(g_i, G_I, AF.Sigmoid, U_I0, U_I1)
                        act_gate(g_u, G_U, AF.Tanh, U_U0, U_U1)
                        act_gate(g_o, G_O, AF.Sigmoid, U_O0, U_O1)

                        if not leaf:
                            # zf = Wf x (no bias; bias folded into the STT).
                            # STT can read only one PSUM operand, so zf is
                            # staged to SBUF (bf16) with a DVE copy.
                            zf = psf.tile([P, 1024], F32, tag="pf", name="zf")[:, :ch]
                            for ko in range(2):
                                for sb in range(nsb):
                                    b0 = sb * 512
                                    w = min(512, ch - b0)
                                    nc.tensor.matmul(
                                        zf[:, b0 : b0 + w],
                                        lhsT=wx_sb[:, G_F, ko, mo],
                                        rhs=xt_t[:, ko, b0 : b0 + w],
                                        start=(ko == 0),
                                        stop=(ko == 1),
                                    )
                            zf_sb = gp.tile([P, 1024], BF16, tag="zf", name="zf_sb")
                            nc.vector.tensor_copy(zf_sb[:, :ch], zf)
                            for fi, u_idx, base in (
                                (0, U_F0, s),
                                (1, U_F1, half + s),
                            ):
                                pt = psf.tile([P, 1024], F32, tag="pf", name="pf")[:, :ch]
                                for ko in range(2):
                                    for sb in range(nsb):
                                        b0 = sb * 512
                                        w = min(512, ch - b0)
                                        nc.tensor.matmul(
                                            pt[:, b0 : b0 + w],
                                            lhsT=wu_sb[:, u_idx, ko, mo],
                                            rhs=h_prev[:, ko, base + b0 : base + b0 + w],
                                            start=(ko == 0),
                                            stop=(ko == 1),
                                        )
                                # fq = (Uf h + bf) + Wf x
                                nc.vector.scalar_tensor_tensor(
                                    out=fq[:, mo, fi, :ch],
                                    in0=pt,
                                    scalar=bs_sb[:, G_F, mo : mo + 1],
                                    in1=zf_sb[:, :ch],
                                    op0=ALU.add,
                                    op1=ALU.add,
                                )
                            # split sigmoid per mo so the t0/t1 muls can
                            # start as soon as their half is ready
                            nc.scalar.activation(
                                f_t[:, mo, :, :ch], fq[:, mo, :, :ch], AF.Sigmoid
                            )

                    # ---- elementwise (both mo at once, 2D APs, bf16 2x) ----
                    # top levels split the chain in half-chunks so the next
                    # level's matmuls start after half the tail latency
                    th = gp.tile([P, 2, 1024], BF16, tag="th", name="th")
                    if leaf:
                        c_ap = c_cur[:, :, s:e]
                        nc.vector.tensor_mul(
                            out=c_ap, in0=g_i[:, :, :ch], in1=g_u[:, :, :ch]
                        )
                        nc.scalar.activation(th[:, :, :ch], c_ap, AF.Tanh)
                        nc.vector.tensor_mul(
                            out=h_cur[:, :, s:e], in0=g_o[:, :, :ch], in1=th[:, :, :ch]
                        )
                    else:
                        iu = gp.tile([P, 2, 1024], BF16, tag="iu", name="iu")
                        nc.vector.tensor_mul(
                            out=iu[:, :, :ch], in0=g_i[:, :, :ch], in1=g_u[:, :, :ch]
                        )
                        t0 = gp.tile([P, 2, 1024], BF16, tag="t0", name="t0")
                        t1 = gp.tile([P, 2, 1024], BF16, tag="t1", name="t1")
                        nhs = 2 if lvl <= LTOP - 2 else 1
                        hw2 = ch // nhs
                        for hs in range(nhs):
                            b0 = hs * hw2
                            b1 = b0 + hw2
                            c_ap = c_cur[:, :, s + b0 : s + b1]
                            nc.vector.tensor_mul(
                                out=t0[:, :, b0:b1],
                                in0=f_t[:, :, 0, b0:b1],
                                in1=c_prev[:, :, s + b0 : s + b1],
                            )
                            nc.vector.tensor_mul(
                                out=t1[:, :, b0:b1],
                                in0=f_t[:, :, 1, b0:b1],
                                in1=c_prev[:, :, half + s + b0 : half + s + b1],
                            )
                            nc.vector.tensor_add(
                                out=c_ap, in0=iu[:, :, b0:b1], in1=t0[:, :, b0:b1]
                            )
                            nc.vector.tensor_add(
                                out=c_ap, in0=c_ap, in1=t1[:, :, b0:b1]
                            )
                            nc.scalar.activation(th[:, :, b0:b1], c_ap, AF.Tanh)
                            nc.vector.tensor_mul(
                                out=h_cur[:, :, s + b0 : s + b1],
                                in0=g_o[:, :, b0:b1],
                                in1=th[:, :, b0:b1],
                            )
                return h_cur, c_cur

            def on_chunk_leaf(ci, c_cur):
                # The U-table (and Wf) DMA is gated behind the third
                # processed leaf chunk (WAW via a 1-elem copy that reads leaf
                # c) so its HBM traffic cannot starve the leaf xt stream.
                if ci == 1:
                    nc.vector.tensor_copy(
                        wu_sb[0:1, 0, 0, 0, 0:1], c_cur[0:1, 0, 0:1]
                    )
                    nc.sync.dma_start(wu_sb[:], wu)
                    nc.sync.dma_start(wx_sb[:, 3:4], wx[:, 3:4])

            # x-weights + biases first: the first matmul needs them
            nc.sync.dma_start(wx_sb[:, 0:3], wx[:, 0:3])
            nc.sync.dma_start(bs_sb[:], bs)

            h_prev = c_prev = None
            for lvl in range(LTOP, CUT - 1, -1):
                h_prev, c_prev = level(
                    lvl, h_prev, c_prev,
                    on_chunk=on_chunk_leaf if lvl == LTOP else None,
                )

            nc.sync.dma_start(hc[:, 0], h_prev)
            nc.sync.dma_start(hc[:, 1], c_prev)
    nc.compile()
    return nc


_CACHE = {}


def _get_program():
    if "nc" not in _CACHE:
        _CACHE["nc"] = _build_program()
    return _CACHE["nc"]


def _core_index_table():
    if "idx" in _CACHE:
        return _CACHE["idx"]
    idx = np.zeros((NCORES, NSLOT), dtype=np.int64)
    for lvl in range(LTOP, CUT - 1, -1):
        n = N_L[lvl]
        rev = _bitrev(lvl - 3)
        start = (1 << lvl) - 1
        for m in range(NCORES):
            pos = m * n + rev
            idx[m, OFF[lvl] : OFF[lvl] + n] = start + pos
    _CACHE["idx"] = idx
    return idx


def _pack_w(mat):
    """[out,in] (256,256) -> [p, ko, mo, m] = W.T[ko*128+p, mo*128+m]."""
    return mat.reshape(2, P, 2, P).transpose(3, 2, 0, 1)


def _sigmoid(x):
    return 1.0 / (1.0 + np.exp(-x))


def _host_node_batch(x, ch_h, ch_c, prm):
    (Wi, bi, Ui, Wo, bo, Uo, Wu, bu, Uu, Wf, bf, Uf) = prm

    def gate(W, b, U):
        return x @ W.T + b + ch_h[:, 0] @ U[0].T + ch_h[:, 1] @ U[1].T

    i = _sigmoid(gate(Wi, bi, Ui))
    o = _sigmoid(gate(Wo, bo, Uo))
    u = np.tanh(gate(Wu, bu, Uu))
    xf = x @ Wf.T + bf
    f0 = _sigmoid(xf + ch_h[:, 0] @ Uf[0].T)
    f1 = _sigmoid(xf + ch_h[:, 1] @ Uf[1].T)
    c = i * u + f0 * ch_c[:, 0] + f1 * ch_c[:, 1]
    h = o * np.tanh(c)
    return h.astype(np.float32), c.astype(np.float32)


def kernel(emb, W_i, b_i, U_i, W_o, b_o, U_o, W_u, b_u, U_u, W_f, b_f, U_f):
    emb = np.asarray(emb, dtype=np.float32)
    f = lambda a: np.asarray(a, dtype=np.float32)
    W_i, b_i, U_i = f(W_i), f(b_i), f(U_i)
    W_o, b_o, U_o = f(W_o), f(b_o), f(U_o)
    W_u, b_u, U_u = f(W_u), f(b_u), f(U_u)
    W_f, b_f, U_f = f(W_f), f(b_f), f(U_f)

    nc = _get_program()
    idx = _core_index_table()

    wx = np.ascontiguousarray(
        np.stack([_pack_w(m) for m in (W_i, W_o, W_u, W_f)], axis=1)
    ).astype(NPBF16)
    wu = np.ascontiguousarray(
        np.stack(
            [
                _pack_w(m)
                for m in (
                    U_i[0], U_i[1], U_o[0], U_o[1],
                    U_u[0], U_u[1], U_f[0], U_f[1],
                )
            ],
            axis=1,
        )
    ).astype(NPBF16)
    bsv = np.ascontiguousarray(
        np.stack([b.reshape(2, P).T for b in (b_i, b_o, b_u, b_f)], axis=1)
    )

    in_maps = []
    npad = NBLK * CHUNK
    for m in range(NCORES):
        xm = emb[idx[m]]  # [NSLOT, 256]
        arr = np.zeros((256, npad), dtype=np.float32)
        arr[:, :NSLOT] = xm.T
        xtc = np.ascontiguousarray(
            arr.reshape(2, P, NBLK, CHUNK).transpose(2, 1, 0, 3)
        ).astype(NPBF16)  # [blk, p, ko, s]
        in_maps.append({"xtb": xtc, "wx": wx, "wu": wu, "bs": bsv})

    kw = {}
    if os.environ.get("KERNEL_TRACE_DIR"):
        kw = {"trace": True, "tmpdir": os.environ["KERNEL_TRACE_DIR"]}
    res = run_bass_kernel_spmd(nc, in_maps, core_ids=list(range(NCORES)), **kw)
    _CACHE["last_res"] = res

    rev = _bitrev(CUT - 3)
    n_cut = 1 << CUT
    h = np.zeros((n_cut, H), dtype=np.float32)
    c = np.zeros((n_cut, H), dtype=np.float32)
    for m in range(NCORES):
        out = np.asarray(res.results[m]["hc"]).astype(np.float32)  # [P,2,2,NOUT]
        pos = m * NOUT + rev
        # feature-major [p, hc, ko, j] -> node-major [j, ko*128+p]
        h[pos] = out[:, 0].transpose(1, 0, 2).reshape(H, NOUT).T
        c[pos] = out[:, 1].transpose(1, 0, 2).reshape(H, NOUT).T

    prm = (W_i, b_i, U_i, W_o, b_o, U_o, W_u, b_u, U_u, W_f, b_f, U_f)
    for lvl in range(CUT - 1, -1, -1):
        n = 1 << lvl
        start = n - 1
        ch_h = h.reshape(n, 2, H)
        ch_c = c.reshape(n, 2, H)
        h, c = _host_node_batch(emb[start : start + n], ch_h, ch_c, prm)

    return h[0], c[0]


# revision 21
# speedup vs baseline: 1.0922x; 1.0042x over previous
"""Bass/Trainium2 kernel for nn_NaryTreeLSTM (binary TreeLSTM over a complete
depth-16 tree, H=D=256, heap/level node order).

Sharding: data-parallel over 8 independent subtrees (core m owns the subtree
rooted at level-3 node m). The device computes levels 15..CUT per core in a
single F-layout (feature-on-partition, nodes-on-free) pipeline; the tiny top
of the tree (2^CUT-1 nodes) is finished on host during the gather/unshard
step (the cross-core combine has to leave the device at level 3 anyway, and
the top levels are latency-bound serial remnants on device).

vs the fp32r baseline: all-bf16 matmul/elementwise datapath (final error
~5e-4, well under the 2e-2 gate; halves HBM traffic and enables fast weight
loads), Wf*x computed once per node and shared between both forget gates
via a fused scalar_tensor_tensor on DVE, wide multi-bank ACTIVATEs (the
scalar engine has no instruction pipelining, so its ~290ns per-instruction
fixed cost dominates), both output-feature halves processed per chunk with
single 2D-AP DVE ops in 2x bf16 mode, per-level chunk-width taper
(1024 -> 512 -> 256 toward the top) with paired chunk ordering so each
level starts after two child chunks, and the latency-bound N-layout tail +
PE transposes are gone entirely (CUT=12).

Node order per level is bit-reversed so the even/odd children of a
contiguous parent chunk are the first/second half of the child level.

Per node (children h_e,h_o / c_e,c_o; x = emb row):
  i = sig(Wi x + bi + Ui0 h_e + Ui1 h_o)      o, u analogous (u: tanh)
  f0 = sig(Wf x + bf + Uf0 h_e),  f1 = sig(Wf x + bf + Uf1 h_o)
  c = i*u + f0*c_e + f1*c_o ;  h = o * tanh(c)
"""

import os

import numpy as np

try:
    import concourse  # noqa: F401
except ImportError:  # pragma: no cover
    import sys

    sys.path.insert(0, "/opt/trn_rl_repo")

import ml_dtypes

import concourse.tile as tile
from concourse import bacc, mybir
from concourse.bass_utils import run_bass_kernel_spmd

F32 = mybir.dt.float32
BF16 = mybir.dt.bfloat16
AF = mybir.ActivationFunctionType
ALU = mybir.AluOpType
NPBF16 = ml_dtypes.bfloat16

DEPTH = 16
H = 256
P = 128
NCORES = 8
LTOP = DEPTH - 1
CUT = int(os.environ.get("TREELSTM_CUT", "12"))  # device: levels 15..CUT
CHUNK = 1024  # two PSUM banks per gate tile; one ACTIVATE per gate per chunk

N_L = {l: 1 << (l - 3) for l in range(CUT, LTOP + 1)}
NSLOT = sum(N_L.values())
OFF = {}
_o = 0
for _l in range(LTOP, CUT - 1, -1):
    OFF[_l] = _o
    _o += N_L[_l]
NOUT = N_L[CUT]
NBLK = (NSLOT + CHUNK - 1) // CHUNK

# weight table gate indices
G_I, G_O, G_U, G_F = 0, 1, 2, 3
U_I0, U_I1, U_O0, U_O1, U_U0, U_U1, U_F0, U_F1 = range(8)


def _bitrev(nbits):
    n = 1 << nbits
    r = np.zeros(n, dtype=np.int64)
    for j in range(n):
        v = 0
        for b in range(nbits):
            if j & (1 << b):
                v |= 1 << (nbits - 1 - b)
        r[j] = v
    return r


def _build_program():
    nc = bacc.Bacc("TRN2", target_bir_lowering=False, debug=False, num_devices=NCORES)
    xtb = nc.dram_tensor("xtb", [NBLK, P, 2, CHUNK], BF16, kind="ExternalInput").ap()
    wx = nc.dram_tensor("wx", [P, 4, 2, 2, P], BF16, kind="ExternalInput").ap()
    wu = nc.dram_tensor("wu", [P, 8, 2, 2, P], BF16, kind="ExternalInput").ap()
    bs = nc.dram_tensor("bs", [P, 4, 2], F32, kind="ExternalInput").ap()
    hc = nc.dram_tensor("hc", [P, 2, 2, NOUT], BF16, kind="ExternalOutput").ap()

    with tile.TileContext(nc) as tc:
        with (
            tc.tile_pool(name="const", bufs=1) as const,
            tc.tile_pool(name="xp", bufs=4) as xp,
            tc.tile_pool(name="fstate", bufs=1) as fstate,
            tc.tile_pool(name="gp", bufs=2) as gp,
            tc.tile_pool(name="psg", bufs=2, space="PSUM") as psg,
            tc.tile_pool(name="psf", bufs=4, space="PSUM") as psf,
        ):
            wx_sb = const.tile([P, 4, 2, 2, P], BF16)
            wu_sb = const.tile([P, 8, 2, 2, P], BF16)
            bs_sb = const.tile([P, 4, 2], F32)

            # chunk width per level: wide at the bulky bottom (fewer, cheaper
            # ACT/DMA instructions), narrow at the top (short serial tails at
            # the level boundaries, which are latency-bound)
            CW = {LTOP: 1024, LTOP - 1: 512, LTOP - 2: 512, LTOP - 3: 256,
                  LTOP - 4: 256}

            def level(lvl, h_prev, c_prev, on_chunk=None):
                n = N_L[lvl]
                cw = CW[lvl]
                leaf = lvl == LTOP
                h_cur = fstate.tile([P, 2, n], BF16, tag=f"h{lvl % 2}", name="h")
                c_cur = fstate.tile([P, 2, n], BF16, tag=f"c{lvl % 2}", name="c")
                half = N_L[lvl + 1] // 2 if not leaf else 0
                nch = (n + cw - 1) // cw
                # paired order: chunk j's parent needs child chunks (j, j+nch/2)
                order = []
                for j in range(nch // 2):
                    order += [j, nch // 2 + j]
                if not order:
                    order = list(range(nch))
                for ci in order:
                    s = ci * cw
                    ch = min(cw, n - s)
                    e = s + ch
                    nsb = (ch + 511) // 512  # 512-wide PSUM sub-banks
                    first = False
                    xt_t = xp.tile([P, 2, 1024], BF16, tag="x", name="x")
                    blk = (OFF[lvl] + s) // CHUNK
                    w0 = (OFF[lvl] + s) % CHUNK
                    nc.sync.dma_start(xt_t[:, :, :ch], xtb[blk][:, :, w0 : w0 + ch])
                    if on_chunk is not None:
                        on_chunk(ci, c_cur)

                    g_i = gp.tile([P, 2, 1024], BF16, tag="gi", name="gi")
                    g_o = gp.tile([P, 2, 1024], BF16, tag="go", name="go")
                    g_u = gp.tile([P, 2, 1024], BF16, tag="gu", name="gu")
                    if not leaf:
                        fq = gp.tile([P, 2, 2, 1024], BF16, tag="fq", name="fq")
                        f_t = gp.tile([P, 2, 2, 1024], BF16, tag="ft", name="ft")

                    for mo in range(2):

                        def gate_mm(pt, g_idx, u0_idx, u1_idx):
                            # ko-outer so the two sub-banks share one
                            # LDWEIGHTS per weight (sb-outer on the very
                            # first chunk: its xt DMA arrives in halves)
                            nmm = 2 if leaf else 6
                            k = 0
                            if first and nsb == 2:
                                for sb in range(2):
                                    b0 = sb * 512
                                    for ko in range(2):
                                        nc.tensor.matmul(
                                            pt[:, b0 : b0 + 512],
                                            lhsT=wx_sb[:, g_idx, ko, mo],
                                            rhs=xt_t[:, ko, b0 : b0 + 512],
                                            start=(ko == 0),
                                            stop=(ko == nmm - 1),
                                        )
                                k = 2
                            else:
                                for ko in range(2):
                                    for sb in range(nsb):
                                        b0 = sb * 512
                                        w = min(512, ch - b0)
                                        nc.tensor.matmul(
                                            pt[:, b0 : b0 + w],
                                            lhsT=wx_sb[:, g_idx, ko, mo],
                                            rhs=xt_t[:, ko, b0 : b0 + w],
                                            start=(k == 0),
                                            stop=(k == nmm - 1),
                                        )
                                    k += 1
                            if not leaf:
                                for u_idx, base in ((u0_idx, s), (u1_idx, half + s)):
                                    for ko in range(2):
                                        for sb in range(nsb):
                                            b0 = sb * 512
                                            w = min(512, ch - b0)
                                            nc.tensor.matmul(
                                                pt[:, b0 : b0 + w],
                                                lhsT=wu_sb[:, u_idx, ko, mo],
                                                rhs=h_prev[
                                                    :, ko, base + b0 : base + b0 + w
                                                ],
                                                start=False,
                                                stop=(k == nmm - 1),
                                            )
                                        k += 1

                        def act_gate(dst, g_idx, func, u0_idx=0, u1_idx=0):
                            pt = psg.tile([P, 1024], F32, tag="ps", name="ps")[:, :ch]
                            gate_mm(pt, g_idx, u0_idx, u1_idx)
                            nc.scalar.activation(
                                dst[:, mo, :ch], pt, func,
                                bias=bs_sb[:, g_idx, mo : mo + 1],
                            )

                        act_gate(g_i, G_I, AF.Sigmoid, U_I0, U_I1)
                        act_gate(g_u, G_U, AF.Tanh, U_U0, U_U1)
                        act_gate(g_o, G_O, AF.Sigmoid, U_O0, U_O1)

                        if not leaf:
                            # zf = Wf x (no bias; bias folded into the STT).
                            # STT can read only one PSUM operand, so zf is
                            # staged to SBUF (bf16) with a DVE copy.
                            zf = psf.tile([P, 512], F32, tag="pf", name="zf")[:, :ch]
                            for ko in range(2):
                                for sb in range(nsb):
                                    b0 = sb * 512
                                    w = min(512, ch - b0)
                                    nc.tensor.matmul(
                                        zf[:, b0 : b0 + w],
                                        lhsT=wx_sb[:, G_F, ko, mo],
                                        rhs=xt_t[:, ko, b0 : b0 + w],
                                        start=(ko == 0),
                                        stop=(ko == 1),
                                    )
                            zf_sb = gp.tile([P, 512], BF16, tag="zf", name="zf_sb")
                            nc.vector.tensor_copy(zf_sb[:, :ch], zf)
                            for fi, u_idx, base in (
                                (0, U_F0, s),
                                (1, U_F1, half + s),
                            ):
                                pt = psf.tile([P, 512], F32, tag="pf", name="pf")[:, :ch]
                                for ko in range(2):
                                    for sb in range(nsb):
                                        b0 = sb * 512
                                        w = min(512, ch - b0)
                                        nc.tensor.matmul(
                                            pt[:, b0 : b0 + w],
                                            lhsT=wu_sb[:, u_idx, ko, mo],
                                            rhs=h_prev[:, ko, base + b0 : base + b0 + w],
                                            start=(ko == 0),
                                            stop=(ko == 1),
                                        )
                                # fq = (Uf h + bf) + Wf x
                                nc.vector.scalar_tensor_tensor(
                                    out=fq[:, mo, fi, :ch],
                                    in0=pt,
                                    scalar=bs_sb[:, G_F, mo : mo + 1],
                                    in1=zf_sb[:, :ch],
                                    op0=ALU.add,
                                    op1=ALU.add,
                                )
                            # split sigmoid per mo so the t0/t1 muls can
                            # start as soon as their half is ready
                            nc.scalar.activation(
                                f_t[:, mo, :, :ch], fq[:, mo, :, :ch], AF.Sigmoid
                            )

                    # ---- elementwise (both mo at once, 2D APs, bf16 2x) ----
                    # top levels split the chain in half-chunks so the next
                    # level's matmuls start after half the tail latency
                    th = gp.tile([P, 2, 1024], BF16, tag="th", name="th")
                    if leaf:
                        c_ap = c_cur[:, :, s:e]
                        nc.vector.tensor_mul(
                            out=c_ap, in0=g_i[:, :, :ch], in1=g_u[:, :, :ch]
                        )
                        nc.scalar.activation(th[:, :, :ch], c_ap, AF.Tanh)
                        nc.vector.tensor_mul(
                            out=h_cur[:, :, s:e], in0=g_o[:, :, :ch], in1=th[:, :, :ch]
                        )
                    else:
                        iu = gp.tile([P, 2, 1024], BF16, tag="iu", name="iu")
                        nc.vector.tensor_mul(
                            out=iu[:, :, :ch], in0=g_i[:, :, :ch], in1=g_u[:, :, :ch]
                        )
                        t0 = gp.tile([P, 2, 1024], BF16, tag="t0", name="t0")
                        t1 = gp.tile([P, 2, 1024], BF16, tag="t1", name="t1")
                        nhs = 4 if lvl == LTOP - 2 else (2 if lvl < LTOP - 2 else 1)
                        hw2 = ch // nhs
                        for hs in range(nhs):
                            b0 = hs * hw2
                            b1 = b0 + hw2
                            c_ap = c_cur[:, :, s + b0 : s + b1]
                            nc.vector.tensor_mul(
                                out=t0[:, :, b0:b1],
                                in0=f_t[:, :, 0, b0:b1],
                                in1=c_prev[:, :, s + b0 : s + b1],
                            )
                            nc.vector.tensor_mul(
                                out=t1[:, :, b0:b1],
                                in0=f_t[:, :, 1, b0:b1],
                                in1=c_prev[:, :, half + s + b0 : half + s + b1],
                            )
                            nc.vector.tensor_add(
                                out=c_ap, in0=iu[:, :, b0:b1], in1=t0[:, :, b0:b1]
                            )
                            nc.vector.tensor_add(
                                out=c_ap, in0=c_ap, in1=t1[:, :, b0:b1]
                            )
                            nc.scalar.activation(th[:, :, b0:b1], c_ap, AF.Tanh)
                            nc.vector.tensor_mul(
                                out=h_cur[:, :, s + b0 : s + b1],
                                in0=g_o[:, :, b0:b1],
                                in1=th[:, :, b0:b1],
                            )
                            if lvl == CUT:
                                # stream the outputs out as they are produced
                                # instead of two big DMAs at the very end
                                nc.sync.dma_start(
                                    hc[:, 1, :, s + b0 : s + b1], c_ap
                                )
                                nc.sync.dma_start(
                                    hc[:, 0, :, s + b0 : s + b1],
                                    h_cur[:, :, s + b0 : s + b1],
                                )
                return h_cur, c_cur

            def on_chunk_leaf(ci, c_cur):
                # The U-table (and Wf) DMA is gated behind the third
                # processed leaf chunk (WAW via a 1-elem copy that reads leaf
                # c) so its HBM traffic cannot starve the leaf xt stream.
                if ci == 1:
                    nc.vector.tensor_copy(
                        wu_sb[0:1, 0, 0, 0, 0:1], c_cur[0:1, 0, 0:1]
                    )
                    nc.sync.dma_start(wu_sb[:], wu)
                    nc.sync.dma_start(wx_sb[:, 3:4], wx[:, 3:4])

            # x-weights + biases first: the first matmul needs them
            nc.sync.dma_start(wx_sb[:, 0:3], wx[:, 0:3])
            nc.sync.dma_start(bs_sb[:], bs)

            h_prev = c_prev = None
            for lvl in range(LTOP, CUT - 1, -1):
                h_prev, c_prev = level(
                    lvl, h_prev, c_prev,
                    on_chunk=on_chunk_leaf if lvl == LTOP else None,
                )
    nc.compile()
    return nc


_CACHE = {}


def _get_program():
    if "nc" not in _CACHE:
        _CACHE["nc"] = _build_program()
    return _CACHE["nc"]


def _core_index_table():
    if "idx" in _CACHE:
        return _CACHE["idx"]
    idx = np.zeros((NCORES, NSLOT), dtype=np.int64)
    for lvl in range(LTOP, CUT - 1, -1):
        n = N_L[lvl]
        rev = _bitrev(lvl - 3)
        start = (1 << lvl) - 1
        for m in range(NCORES):
            pos = m * n + rev
            idx[m, OFF[lvl] : OFF[lvl] + n] = start + pos
    _CACHE["idx"] = idx
    return idx


def _pack_w(mat):
    """[out,in] (256,256) -> [p, ko, mo, m] = W.T[ko*128+p, mo*128+m]."""
    return mat.reshape(2, P, 2, P).transpose(3, 2, 0, 1)


def _sigmoid(x):
    return 1.0 / (1.0 + np.exp(-x))


def _host_node_batch(x, ch_h, ch_c, prm):
    (Wi, bi, Ui, Wo, bo, Uo, Wu, bu, Uu, Wf, bf, Uf) = prm

    def gate(W, b, U):
        return x @ W.T + b + ch_h[:, 0] @ U[0].T + ch_h[:, 1] @ U[1].T

    i = _sigmoid(gate(Wi, bi, Ui))
    o = _sigmoid(gate(Wo, bo, Uo))
    u = np.tanh(gate(Wu, bu, Uu))
    xf = x @ Wf.T + bf
    f0 = _sigmoid(xf + ch_h[:, 0] @ Uf[0].T)
    f1 = _sigmoid(xf + ch_h[:, 1] @ Uf[1].T)
    c = i * u + f0 * ch_c[:, 0] + f1 * ch_c[:, 1]
    h = o * np.tanh(c)
    return h.astype(np.float32), c.astype(np.float32)


def kernel(emb, W_i, b_i, U_i, W_o, b_o, U_o, W_u, b_u, U_u, W_f, b_f, U_f):
    emb = np.asarray(emb, dtype=np.float32)
    f = lambda a: np.asarray(a, dtype=np.float32)
    W_i, b_i, U_i = f(W_i), f(b_i), f(U_i)
    W_o, b_o, U_o = f(W_o), f(b_o), f(U_o)
    W_u, b_u, U_u = f(W_u), f(b_u), f(U_u)
    W_f, b_f, U_f = f(W_f), f(b_f), f(U_f)

    nc = _get_program()
    idx = _core_index_table()

    wx = np.ascontiguousarray(
        np.stack([_pack_w(m) for m in (W_i, W_o, W_u, W_f)], axis=1)
    ).astype(NPBF16)
    wu = np.ascontiguousarray(
        np.stack(
            [
                _pack_w(m)
                for m in (
                    U_i[0], U_i[1], U_o[0], U_o[1],
                    U_u[0], U_u[1], U_f[0], U_f[1],
                )
            ],
            axis=1,
        )
    ).astype(NPBF16)
    bsv = np.ascontiguousarray(
        np.stack([b.reshape(2, P).T for b in (b_i, b_o, b_u, b_f)], axis=1)
    )

    in_maps = []
    npad = NBLK * CHUNK
    for m in range(NCORES):
        xm = emb[idx[m]]  # [NSLOT, 256]
        arr = np.zeros((256, npad), dtype=np.float32)
        arr[:, :NSLOT] = xm.T
        xtc = np.ascontiguousarray(
            arr.reshape(2, P, NBLK, CHUNK).transpose(2, 1, 0, 3)
        ).astype(NPBF16)  # [blk, p, ko, s]
        in_maps.append({"xtb": xtc, "wx": wx, "wu": wu, "bs": bsv})

    kw = {}
    if os.environ.get("KERNEL_TRACE_DIR"):
        kw = {"trace": True, "tmpdir": os.environ["KERNEL_TRACE_DIR"]}
    res = run_bass_kernel_spmd(nc, in_maps, core_ids=list(range(NCORES)), **kw)
    _CACHE["last_res"] = res

    rev = _bitrev(CUT - 3)
    n_cut = 1 << CUT
    h = np.zeros((n_cut, H), dtype=np.float32)
    c = np.zeros((n_cut, H), dtype=np.float32)
    for m in range(NCORES):
        out = np.asarray(res.results[m]["hc"]).astype(np.float32)  # [P,2,2,NOUT]
        pos = m * NOUT + rev
        # feature-major [p, hc, ko, j] -> node-major [j, ko*128+p]
        h[pos] = out[:, 0].transpose(1, 0, 2).reshape(H, NOUT).T
        c[pos] = out[:, 1].transpose(1, 0, 2).reshape(H, NOUT).T

    prm = (W_i, b_i, U_i, W_o, b_o, U_o, W_u, b_u, U_u, W_f, b_f, U_f)
    for lvl in range(CUT - 1, -1, -1):
        n = 1 << lvl
        start = n - 1
        ch_h = h.reshape(n, 2, H)
        ch_c = c.reshape(n, 2, H)
        h, c = _host_node_batch(emb[start : start + n], ch_h, ch_c, prm)

    return h[0], c[0]
